# revision 1
# baseline (speedup 1.0000x reference)
"""Weighted-MAE loss (nn_MAELoss) on 8 Trainium2 NeuronCores.

reference:  w = bucket-weights(y_true) via thresholds log1p(5/25/50),
            loss = sum(w * |y_true - y_pred|) / sum(w)

Strategy: data-parallel over the batch dim (8 shards of 8 batches), each
core reduces its [128, 15360] shard to a handful of per-partition fp32
accumulators; the host combines them in float64 and divides.

Per-core dataflow (near the HBM roofline, ~44us of DMA for 15.7MB/core):
  DMA   : y_true/y_pred live as full resident SBUF buffers, streamed in
          variable-size column chunks (small head/tail chunks shorten
          pipeline fill/drain; Tile tracks range-level deps).
  GPSIMD: d = yt - yp into small rotating tiles (feeds E2 only).
  DVE   : two fused custom ops per work span:
            E1 = sum(((yt>=T1) + 0.2/29.8) * |yt - yp|)   (diff fused in)
            E2 = sum(((yt>=T2) + r*(yt>=T3)) * |d|),  r = 17500/2470
          so sum(w*|diff|) = 29.8*E1 + 2470*E2 with no separate abs pass
          and no sum|diff| accumulator; plus part of count(yt>=T3) as a
          stock tensor_scalar (2x perf mode, exact is_ge).
  ACT   : sign-counts for T1, T2 (and mid-stream T3 spans); biases sit one
          ulp below each threshold so exact hits count like the reference.
All engines stay below the ~44us DMA stream; the host combines the
per-partition partials in float64.
"""

import os
import sys

import numpy as np

# concourse ships on the default sys.path in the target containers; fall back
# to the known staging locations if not.
try:
    import concourse  # noqa: F401
except ImportError:  # pragma: no cover
    for _p in ("/root/.axon_site/_ro/trn_rl_repo", "/opt/trn_rl_repo"):
        if os.path.isdir(_p) and _p not in sys.path:
            sys.path.append(_p)

from contextlib import ExitStack
from operator import add

import concourse.bacc as bacc
import concourse.tile as tile
from concourse import mybir
from concourse.bass_utils import run_bass_kernel_spmd
import concourse.dve_ops as dve_ops
from concourse.dve_ops import DveOp
from concourse.dve_spec import (
    C0,
    C1,
    C2,
    Spec,
    Src0,
    Src1,
    Zero,
    _has_src1,
    lower,
    maxx,
)
from concourse.dve_uop import DveOpSpec

# ----------------------------------------------------------------- problem
N_CORES = 8
B, C, T, H, W = 64, 1, 15, 128, 128
SHARD_B = B // N_CORES
P = 128
F = SHARD_B * C * T * H * W // P  # 15360
N_TOTAL = B * C * T * H * W      # 15728640

THR1 = float(np.float32(np.log1p(5.0)))
THR2 = float(np.float32(np.log1p(25.0)))
THR3 = float(np.float32(np.log1p(50.0)))
W_BASE = 0.2          # bucket-0 weight
DW1 = 29.8            # 30 - 0.2
DW2 = 2470.0          # 2500 - 30
DW3 = 17500.0         # 20000 - 2500
LAM1 = float(np.float32(W_BASE / DW1))   # folds 0.2*sum|d| into E1
RATIO32 = float(np.float32(DW3 / DW2))   # folds the T3 level into E2

# Granularities are decoupled (Tile tracks range-level deps):
#   DMA chunks: large for bandwidth, small at head/tail for fill/drain
#   work spans (sub + products): fine, so the GPSIMD->DVE chain pipelines
#   cnt3 / sign spans: wider, amortizing per-op fixed costs
CHUNKS = [480, 480, 960] + [1920] * 6 + [480, 480, 240, 240, 240, 240]
WORK = [480, 480] + [960] * 13 + [480, 480, 240, 240, 240, 240]
CNT3S = [480, 480, 960] + [1920] * 6 + [480, 480, 480, 480]
# cnt3 engine per span: DVE (2x tensor_scalar) where the DVE otherwise
# idles (early fill, short tail), ACT for the middle spans, balancing both
# engines' end-to-end spans under the DMA floor
CNT3_ON_ACT = [False, False, False, False, False, False,
               False, True, True, False, False, False, False]
SIGNS = [480, 480, 960] + [1920] * 6 + [480, 480, 480, 480]
# per-span engine choice for the T1/T2 counts (True = ACT sign op,
# False = DVE 2x tensor_scalar is_ge)
SIGN1_ON_ACT = [True] * len(SIGNS)
SIGN2_ON_ACT = [True] * len(SIGNS)
for _l in (CHUNKS, WORK, CNT3S, SIGNS):
    assert sum(_l) == F
NW, NC3, NSG = len(WORK), len(CNT3S), len(SIGNS)
assert len(CNT3_ON_ACT) == NC3

# accumulator slot layout (each (quantity, span) has a slot in BOTH the
# DVE and ACT regions; the host reads only the assigned engine's slot):
#   acc_dve[:, 2s+0/1] = E1_s / E2_s (work span s)
#   acc_dve[:, 2*NW + g] / [2*NW + NSG + g] = count1_g / count2_g (is_ge)
#   acc_dve[:, 2*NW + 2*NSG + c] = cnt3_c (is_ge)
#   acc_act[:, 2g+0/1] = sum sign(yt-T1/T2) over sign span g
#   acc_act[:, 2*NSG + c] = sum sign(yt-T3) over cnt3 span c
ND = 2 * NW + 2 * NSG + NC3
NA = 2 * NSG + NC3

# ------------------------------------------------------- custom DVE ops
_neg = Zero - Src1
_absd = maxx(Src1, _neg)          # |in1|          (in1 = precomputed d)
_absdiff = maxx(Src0 - Src1, Src1 - Src0)  # |in0 - in1|  (diff fused in)


def _accum_ref(body_fn):
    def _r(in0, in1, s0, s1, imm2):
        b = body_fn(
            in0.astype(np.float32), None if in1 is None else in1.astype(np.float32),
            s0, s1, imm2,
        ).astype(np.float32)
        return b, b.reshape(b.shape[0], -1).sum(axis=-1, keepdims=True).astype(np.float32)
    return _r


def _register_op(name: str, spec: Spec) -> DveOp:
    for op in dve_ops.OPS:
        if op.name == name:
            return op
    row = dve_ops._CUSTOM_DVE_ROW_BASE + len(dve_ops.OPS)
    assert row < 0x20, "custom-DVE row overflow"
    shas = {}
    for ver in ("v3", "v4"):
        try:
            tmp = DveOpSpec(
                name=name, opcode=row, uops=lower(spec, ver=ver),
                rd1_en=_has_src1(spec),
            )
            shas[ver] = tmp.sha(ver)
        except Exception:
            pass
    op = DveOp(name, spec, subdim=False, uops_sha=shas)
    dve_ops.OPS.append(op)
    dve_ops._SUB_OPCODE_FOR_NAME[name] = row
    dve_ops.CUSTOM_DVE_SPECS[name] = spec
    return op


# out = ((in0 >= s0) + s1) * |in0 - in1| ; accum_out = sum(out)
# diff+abs fused in (7 ALU stages) -> no dependency on the GPSIMD subtract
MASK1L = _register_op(
    "WMAE_MASK1LD_ANT",
    Spec(body=((Src0 >= C0) + C1) * _absdiff, accum=add, accum_init=Zero,
         reference=_accum_ref(
             lambda a, b, s0, s1, i2: ((a >= s0) + s1) * np.abs(a - b))),
)
# out = ((in0 >= s0) + imm2*(in0 >= s1)) * |in1| ; accum_out = sum(out)
MASK2 = _register_op(
    "WMAE_MASK2_ANT",
    Spec(body=((Src0 >= C0) + C2 * (Src0 >= C1)) * _absd,
         accum=add, accum_init=Zero,
         reference=_accum_ref(
             lambda a, b, s0, s1, i2: ((a >= s0) + i2 * (a >= s1)) * np.abs(b))),
)

_STATE: dict = {}


def _build():
    """Build + schedule the Bass module once per process."""
    if "nc" in _STATE:
        return _STATE["nc"]
    f32 = mybir.dt.float32
    nc = bacc.Bacc("TRN2", target_bir_lowering=False, debug=False,
                   enable_asserts=False)
    yt_d = nc.dram_tensor("y_true", [P, F], f32, kind="ExternalInput").ap()
    yp_d = nc.dram_tensor("y_pred", [P, F], f32, kind="ExternalInput").ap()
    out_d = nc.dram_tensor("partials", [P, ND + NA], f32,
                           kind="ExternalOutput").ap()

    with tile.TileContext(nc) as tc, ExitStack() as ctx:
        big_pool = ctx.enter_context(tc.tile_pool(name="big", bufs=1))
        mid_pool = ctx.enter_context(tc.tile_pool(name="mid", bufs=4))
        junk_pool = ctx.enter_context(tc.tile_pool(name="junk", bufs=1))
        acc_pool = ctx.enter_context(tc.tile_pool(name="acc", bufs=1))

        yt = big_pool.tile([P, F], f32, tag="yt")
        yp = big_pool.tile([P, F], f32, tag="yp")

        acc = acc_pool.tile([P, ND + NA], f32, tag="acc")
        acc_dve = acc[:, 0:ND]
        acc_act = acc[:, ND:ND + NA]

        # sign(y + bias) counts y >= THR; bias = -(one ulp below THR) so an
        # exact threshold hit lands at +ulp (counted high, matching the
        # reference's `y < THR` branch) instead of sign(0) = 0 (half-count)
        def _below(t):
            return float(np.nextafter(np.float32(t), np.float32(0.0)))

        bias1 = acc_pool.tile([P, 1], f32, tag="bias1")
        nc.vector.memset(bias1[:], -_below(THR1))
        bias2 = acc_pool.tile([P, 1], f32, tag="bias2")
        nc.vector.memset(bias2[:], -_below(THR2))
        bias3 = acc_pool.tile([P, 1], f32, tag="bias3")
        nc.vector.memset(bias3[:], -_below(THR3))

        FS_MAX = max(max(WORK), max(CNT3S), max(SIGNS))
        junk1 = junk_pool.tile([P, FS_MAX], f32, tag="junk1")
        junk2 = junk_pool.tile([P, FS_MAX], f32, tag="junk2")
        junk3 = junk_pool.tile([P, FS_MAX], f32, tag="junk3")
        junks = junk_pool.tile([P, FS_MAX], f32, tag="junks")

        # 1-element dummy Sign pulls the ACT table load into the DMA fill
        nc.scalar.activation(junks[:, 0:1], bias1[:],
                             mybir.ActivationFunctionType.Sign, bias=bias2[:])

        def spans_of(sizes):
            out, c = [], 0
            for fs in sizes:
                out.append((c, c + fs))
                c += fs
            return out

        chunk_sp = spans_of(CHUNKS)
        work_sp = spans_of(WORK)
        cnt3_sp = spans_of(CNT3S)
        sign_sp = spans_of(SIGNS)

        # bucket each op by the DMA chunk that completes its input range
        def ready_idx(end, only_yt):
            # chunk index after which [0, end) of yt (and yp unless only_yt)
            # has landed; DMAs are issued yt-chunk then yp-chunk, in order
            for i, (a, b) in enumerate(chunk_sp):
                if b >= end:
                    return i
            raise AssertionError

        buckets = [[] for _ in CHUNKS]
        for s, (a, b) in enumerate(work_sp):
            buckets[ready_idx(b, False)].append(("work", s, a, b))
        for s, (a, b) in enumerate(cnt3_sp):
            buckets[ready_idx(b, True)].append(("cnt3", s, a, b))
        for s, (a, b) in enumerate(sign_sp):
            buckets[ready_idx(b, True)].append(("sign", s, a, b))

        d_tiles: dict = {}
        # MASK2 products are emitted one DMA-chunk late: their GPSIMD sub
        # runs during the previous chunk's window, so they are ready when
        # they reach the head of the DVE's in-order queue (no head blocking)
        for ci in range(len(chunk_sp) + 1):
            if ci < len(chunk_sp):
                ca, cb = chunk_sp[ci]
                nc.sync.dma_start(yt[:, ca:cb], yt_d[:, ca:cb])
                nc.sync.dma_start(yp[:, ca:cb], yp_d[:, ca:cb])
            kinds = ("work_sub", "cnt3", "sign", "work_prod1", "work_prod2")
            for kind in kinds:
                bi = ci - 1 if kind == "work_prod2" else ci
                if not (0 <= bi < len(chunk_sp)):
                    continue
                for item in buckets[bi]:
                    k, s, a, b = item
                    fs = b - a
                    yt_s, yp_s = yt[:, a:b], yp[:, a:b]
                    if k == "work" and kind == "work_sub":
                        d = mid_pool.tile([P, max(WORK)], f32, tag="d")
                        d_tiles[s] = d
                        nc.gpsimd.tensor_sub(d[:, :fs], yt_s, yp_s)
                    elif k == "work" and kind == "work_prod1":
                        # diff fused into the op -> depends only on the DMAs
                        nc.vector._custom_dve(
                            MASK1L, out=junk1[:, :fs], in0=yt_s, in1=yp_s,
                            s0=THR1, s1=LAM1,
                            accum_out=acc_dve[:, 2 * s:2 * s + 1],
                        )
                    elif k == "work" and kind == "work_prod2":
                        # reads d from GPSIMD; emitted one chunk late so it
                        # is ready at the head of the DVE queue
                        nc.vector._custom_dve(
                            MASK2, out=junk2[:, :fs], in0=yt_s,
                            in1=d_tiles[s][:, :fs],
                            s0=THR2, s1=THR3, imm2=RATIO32,
                            accum_out=acc_dve[:, 2 * s + 1:2 * s + 2],
                        )
                    elif k == "cnt3" and kind == "cnt3":
                        if CNT3_ON_ACT[s]:
                            q = 2 * NSG + s
                            nc.scalar.activation(
                                junk3[:, :fs], yt_s,
                                mybir.ActivationFunctionType.Sign,
                                bias=bias3[:],
                                accum_out=acc_act[:, q:q + 1],
                            )
                        else:
                            q = 2 * NW + 2 * NSG + s
                            nc.vector.tensor_scalar(
                                junk3[:, :fs], yt_s, THR3, 0.0,
                                mybir.AluOpType.is_ge, mybir.AluOpType.add,
                                accum_out=acc_dve[:, q:q + 1],
                            )
                    elif k == "sign" and kind == "sign":
                        for ki, (on_act, bias, thr) in enumerate(
                                ((SIGN1_ON_ACT[s], bias1, THR1),
                                 (SIGN2_ON_ACT[s], bias2, THR2))):
                            if on_act:
                                nc.scalar.activation(
                                    junks[:, :fs], yt_s,
                                    mybir.ActivationFunctionType.Sign,
                                    bias=bias[:],
                                    accum_out=acc_act[:, 2 * s + ki:
                                                      2 * s + ki + 1],
                                )
                            else:
                                q = 2 * NW + ki * NSG + s
                                nc.vector.tensor_scalar(
                                    junk3[:, :fs], yt_s, thr, 0.0,
                                    mybir.AluOpType.is_ge,
                                    mybir.AluOpType.add,
                                    accum_out=acc_dve[:, q:q + 1],
                                )

        nc.sync.dma_start(out_d[:], acc[:])

    nc.compile()
    _STATE["nc"] = nc
    return nc


def _run_device(y_pred: np.ndarray, y_true: np.ndarray, **kw):
    nc = _build()
    y_pred = np.asarray(y_pred, dtype=np.float32).reshape(B, -1)
    y_true = np.asarray(y_true, dtype=np.float32).reshape(B, -1)
    in_maps = []
    for c in range(N_CORES):
        sl = slice(c * SHARD_B, (c + 1) * SHARD_B)
        in_maps.append({
            "y_true": np.ascontiguousarray(y_true[sl]).reshape(P, F),
            "y_pred": np.ascontiguousarray(y_pred[sl]).reshape(P, F),
        })
    return run_bass_kernel_spmd(nc, in_maps, list(range(N_CORES)), **kw)


def _finalize(results) -> np.ndarray:
    e1 = e2 = 0.0
    cnt1 = cnt2 = cnt3 = 0.0
    for c in range(N_CORES):
        part = results[c]["partials"].astype(np.float64)
        dve = part[:, 0:2 * NW].reshape(P, NW, 2)
        act = part[:, ND:ND + 2 * NSG].reshape(P, NSG, 2)
        act3 = part[:, ND + 2 * NSG:ND + NA]
        e1 += dve[:, :, 0].sum()
        e2 += dve[:, :, 1].sum()
        for s, fs in enumerate(CNT3S):
            if CNT3_ON_ACT[s]:
                cnt3 += (act3[:, s].sum() + P * fs) / 2.0
            else:
                cnt3 += part[:, 2 * NW + 2 * NSG + s].sum()
        for s, fs in enumerate(SIGNS):
            n_el = P * fs
            # ACT slots hold sum(sign): count_ge = (sum(sign) + n_elems)/2;
            # DVE slots hold the is_ge count directly
            if SIGN1_ON_ACT[s]:
                cnt1 += (act[:, s, 0].sum() + n_el) / 2.0
            else:
                cnt1 += part[:, 2 * NW + s].sum()
            if SIGN2_ON_ACT[s]:
                cnt2 += (act[:, s, 1].sum() + n_el) / 2.0
            else:
                cnt2 += part[:, 2 * NW + NSG + s].sum()
    sum_wad = DW1 * e1 + DW2 * e2
    sum_w = W_BASE * N_TOTAL + DW1 * cnt1 + DW2 * cnt2 + DW3 * cnt3
    return np.array(sum_wad / sum_w, dtype=np.float32)


def kernel(y_pred: np.ndarray, y_true: np.ndarray) -> np.ndarray:
    try:
        res = _run_device(y_pred, y_true)
    except Exception:
        # transient device-state failures have been observed; retry once
        import time as _time
        _time.sleep(2.0)
        res = _run_device(y_pred, y_true)
    return _finalize(res.results)



# revision 5
# speedup vs baseline: 1.4420x; 1.4420x over previous
"""Weighted-MAE loss (nn_MAELoss) on 8 Trainium2 NeuronCores.

reference:  w = bucket-weights(y_true) via thresholds log1p(5/25/50),
            loss = sum(w * |y_true - y_pred|) / sum(w)

Strategy: data-parallel over the batch dim (8 shards of 8 batches). Inputs
are cast to bf16 on the host (rel err ~2.7e-4, far inside the 2e-2 gate),
halving HBM traffic: 7.86 MB/core -> ~21.8us DMA floor at 360 GB/s.

Per-core dataflow (all stock ops; every engine stays under the DMA floor):
  DMA   : yt/yp stream into full resident SBUF buffers in column chunks.
  DVE   : per chunk, three tensor_scalar is_ge passes build the bf16
          threshold masks m1/m2/m3 (4x perf mode, ~0.26 ns/col) with the
          bucket counts falling out of accum_out for free; most chunks'
          d = yt - yp subtracts also run here (2x perf mode).
  Pool  : the subtracts of the three largest chunks (load balancing).
  ACT   : absd = Abs(d) with accum_out giving sum|d| per chunk.
  PE    : per 40-col microtile, matmul with stationary = [m1|m2|m3]
          (120 cols) and moving = absd (40 cols), accumulated into two
          PSUM banks. psum[40k+i, j] accumulates sum_p m_k[p,i]*absd[p,j];
          the host sums the three 40-wide diagonal bands to get
          S_k = sum(m_k * |d|) -- no elementwise product pass needed.
The host combines counts, S0..S3 in float64 and divides.
"""

import os
import sys

import numpy as np

try:
    import concourse  # noqa: F401
except ImportError:  # pragma: no cover
    for _p in ("/root/.axon_site/_ro/trn_rl_repo", "/opt/trn_rl_repo"):
        if os.path.isdir(_p) and _p not in sys.path:
            sys.path.append(_p)

from contextlib import ExitStack

import concourse.bacc as bacc
import concourse.tile as tile
from concourse import mybir
from concourse.bass_utils import run_bass_kernel_spmd

# ----------------------------------------------------------------- problem
N_CORES = 8
B, C, T, H, W = 64, 1, 15, 128, 128
SHARD_B = B // N_CORES
P = 128
F = SHARD_B * C * T * H * W // P  # 15360
N_TOTAL = B * C * T * H * W      # 15728640

TW = 40                   # microtile width (3*TW = 120 <= 128 stationary)
NT = F // TW              # 384 microtiles
assert NT * TW == F

THR1 = float(np.float32(np.log1p(5.0)))
THR2 = float(np.float32(np.log1p(25.0)))
THR3 = float(np.float32(np.log1p(50.0)))
W_BASE = 0.2
DW1, DW2, DW3 = 29.8, 2470.0, 17500.0

# column chunks in units of TW-col microtiles (sum = NT): small head chunks
# shorten pipeline fill, small tail chunks shorten drain
CHUNKS_T = [8, 12, 16, 24, 32, 40, 48, 48, 48, 40, 32, 20, 12, 4]
assert sum(CHUNKS_T) == NT
NCH = len(CHUNKS_T)
# chunks whose d = yt - yp runs on GPSIMD instead of DVE (the big middle
# ones; keeps DVE below the DMA floor)
SUB_ON_POOL = {6, 7, 8}
# psum bank boundary (in microtiles): bank0 = tiles [0, PSPLIT), shipped
# mid-stream; bank1 = the rest, shipped in the drain
PSPLIT = 192

ND = 4 * NCH  # acc slots: (c1, c2, c3, s0) per chunk

_STATE: dict = {}


def _build():
    if "nc" in _STATE:
        return _STATE["nc"]
    f32 = mybir.dt.float32
    bf16 = mybir.dt.bfloat16
    A = mybir.AluOpType
    nc = bacc.Bacc("TRN2", target_bir_lowering=False, debug=False,
                   enable_asserts=False)
    yt_d = nc.dram_tensor("y_true", [P, NT, TW], bf16, kind="ExternalInput").ap()
    yp_d = nc.dram_tensor("y_pred", [P, NT, TW], bf16, kind="ExternalInput").ap()
    acc_d = nc.dram_tensor("partials", [P, ND], f32, kind="ExternalOutput").ap()
    prod_d = nc.dram_tensor("prods", [3 * TW, 2 * TW], f32,
                            kind="ExternalOutput").ap()

    with tile.TileContext(nc) as tc, ExitStack() as ctx:
        big_pool = ctx.enter_context(tc.tile_pool(name="big", bufs=1))
        d_pool = ctx.enter_context(tc.tile_pool(name="d", bufs=3))
        a_pool = ctx.enter_context(tc.tile_pool(name="a", bufs=3))
        acc_pool = ctx.enter_context(tc.tile_pool(name="acc", bufs=1))
        ps_pool = ctx.enter_context(tc.psum_pool(name="ps", bufs=1))

        yt = big_pool.tile([P, NT, TW], bf16, tag="yt")
        yp = big_pool.tile([P, NT, TW], bf16, tag="yp")
        masks = big_pool.tile([P, NT, 3, TW], bf16, tag="masks")
        acc = acc_pool.tile([P, ND], f32, tag="acc")
        prodsb = acc_pool.tile([3 * TW, 2 * TW], f32, tag="prodsb")
        psum0 = ps_pool.tile([3 * TW, TW], f32, tag="ps0")
        psum1 = ps_pool.tile([3 * TW, TW], f32, tag="ps1")

        CH_MAX = max(CHUNKS_T)
        t0 = 0
        for ci, nt in enumerate(CHUNKS_T):
            t1 = t0 + nt
            yt_s = yt[:, t0:t1, :]
            yp_s = yp[:, t0:t1, :]
            nc.sync.dma_start(yt_s, yt_d[:, t0:t1, :])
            nc.sync.dma_start(yp_s, yp_d[:, t0:t1, :])

            # d = yt - yp (2x perf mode on DVE; big chunks on GPSIMD)
            d = d_pool.tile([P, CH_MAX, TW], bf16, tag="d")
            d_s = d[:, :nt, :]
            if ci in SUB_ON_POOL:
                nc.gpsimd.tensor_sub(d_s, yt_s, yp_s)
            else:
                nc.vector.tensor_sub(d_s, yt_s, yp_s)

            # masks (4x perf mode) + counts via accum_out
            for k, thr in enumerate((THR1, THR2, THR3)):
                # with accum_out, op1 is the reduction op: accum = sum(mask)
                nc.vector.tensor_scalar(
                    masks[:, t0:t1, k, :], yt_s, thr, 0.0,
                    A.is_ge, A.add,
                    accum_out=acc[:, 4 * ci + k:4 * ci + k + 1])

            # absd = |d| on ACT, sum|d| via accum
            absd = a_pool.tile([P, CH_MAX, TW], bf16, tag="absd")
            absd_s = absd[:, :nt, :]
            nc.scalar.activation(absd_s, d_s,
                                 mybir.ActivationFunctionType.Abs,
                                 accum_out=acc[:, 4 * ci + 3:4 * ci + 4])

            # PE: accumulate sum_p m_k[p,i]*absd[p,j] into psum bands
            for tt in range(t0, t1):
                ps = psum0 if tt < PSPLIT else psum1
                first = tt == 0 or tt == PSPLIT
                last = tt == PSPLIT - 1 or tt == NT - 1
                nc.tensor.matmul(
                    ps[:, :],
                    masks[:, tt, :, :],       # [P, 3*TW] stationary
                    absd[:, tt - t0, :],      # [P, TW] moving
                    start=first, stop=last)
                if tt == PSPLIT - 1:
                    nc.vector.tensor_copy(prodsb[:, 0:TW], psum0[:, :])
                    nc.sync.dma_start(prod_d[:, 0:TW], prodsb[:, 0:TW])
            t0 = t1

        nc.scalar.activation(prodsb[:, TW:2 * TW], psum1[:, :],
                             mybir.ActivationFunctionType.Copy)
        nc.sync.dma_start(acc_d[:], acc[:])
        nc.sync.dma_start(prod_d[:, TW:2 * TW], prodsb[:, TW:2 * TW])

    nc.compile()
    _STATE["nc"] = nc
    return nc


def _run_device(y_pred: np.ndarray, y_true: np.ndarray, **kw):
    import ml_dtypes
    nc = _build()
    y_pred = np.asarray(y_pred, dtype=np.float32).reshape(B, -1)
    y_true = np.asarray(y_true, dtype=np.float32).reshape(B, -1)
    in_maps = []
    for c in range(N_CORES):
        sl = slice(c * SHARD_B, (c + 1) * SHARD_B)
        in_maps.append({
            "y_true": np.ascontiguousarray(y_true[sl]).astype(
                ml_dtypes.bfloat16).reshape(P, NT, TW),
            "y_pred": np.ascontiguousarray(y_pred[sl]).astype(
                ml_dtypes.bfloat16).reshape(P, NT, TW),
        })
    return run_bass_kernel_spmd(nc, in_maps, list(range(N_CORES)), **kw)


def _finalize(results) -> np.ndarray:
    c1 = c2 = c3 = s0 = 0.0
    s_band = np.zeros(3, dtype=np.float64)
    for c in range(N_CORES):
        part = results[c]["partials"].astype(np.float64)
        sl = part.sum(axis=0).reshape(NCH, 4)
        c1 += sl[:, 0].sum()
        c2 += sl[:, 1].sum()
        c3 += sl[:, 2].sum()
        s0 += sl[:, 3].sum()
        prods = results[c]["prods"].astype(np.float64)  # [3*TW, 2*TW]
        for k in range(3):
            band = prods[k * TW:(k + 1) * TW, :]
            s_band[k] += np.trace(band[:, 0:TW]) + np.trace(band[:, TW:2 * TW])
    num = W_BASE * s0 + DW1 * s_band[0] + DW2 * s_band[1] + DW3 * s_band[2]
    den = W_BASE * N_TOTAL + DW1 * c1 + DW2 * c2 + DW3 * c3
    return np.array(num / den, dtype=np.float32)


def kernel(y_pred: np.ndarray, y_true: np.ndarray) -> np.ndarray:
    try:
        res = _run_device(y_pred, y_true)
    except Exception:
        import time as _time
        _time.sleep(2.0)
        res = _run_device(y_pred, y_true)
    return _finalize(res.results)
